# revision 1
# baseline (speedup 1.0000x reference)
"""GPT-2 (L=8, D=1024, H=16, V=50257, B=4, T=1024) forward on 8 TRN2 NeuronCores.

Sharding: core c handles batch b=c//2, sequence half h=c%2 (512 tokens).
Weights replicated (bf16). Per layer, K/V for the half-sequence are exchanged
between the two cores of a batch-pair with an AllGather, so every core attends
over the full 1024-token causal context for its own 512 queries.

Activation layout on-chip: x is kept transposed, [d (8x128 partitions), tok],
so every projection matmul uses weights as the stationary operand and never
needs an activation transpose. V is produced in [tok, d] layout directly, and
augmented with a ones-column per head so the AV matmul also produces the
softmax denominators (V_aug is [tok, 16*65]).
"""

import os
import sys
import types
import contextlib

import numpy as np
import ml_dtypes

import concourse.bass as bass
import concourse.mybir as mybir
import concourse.tile as tile
from concourse import bacc
from concourse.bass_utils import run_bass_kernel_spmd

f32 = mybir.dt.float32
bf16 = mybir.dt.bfloat16
AF = mybir.ActivationFunctionType
OP = mybir.AluOpType

L, D, H, V, DFF = 8, 1024, 16, 50257, 4096
HS = D // H          # 64
B, T = 4, 1024
TPC = 512            # tokens per core
P = 128
DC = D // P          # 8 d-chunks
FC = DFF // P        # 32 dff-chunks
NVC = (V + 511) // 512   # 99 vocab chunks
EPS = 1e-5

K_SZ = DC * P * TPC            # K staging elems per core
V_SZ = 4 * P * (H * (HS + 1))  # V_aug staging elems per core (4 tok chunks x 128 x 1040)
KV_SZ = K_SZ + V_SZ
VW = H * (HS + 1)              # 1040

LAST_EXEC_NS = None
_CACHE = {}


def _install_ntff_hook():
    """Provide antenv.axon_hooks if the image lacks it, so trace=True works."""
    try:
        import antenv
        try:
            from antenv import axon_hooks  # noqa: F401
            return
        except ImportError:
            pass
        hooks_mod = types.ModuleType("antenv.axon_hooks")
        _hook = [None]
        hooks_mod.set_axon_ntff_profile_hook = lambda h: _hook.__setitem__(0, h)
        hooks_mod.get_axon_ntff_profile_hook = lambda: _hook[0]
        sys.modules["antenv.axon_hooks"] = hooks_mod
        antenv.axon_hooks = hooks_mod
        from trn_agent_boot.trn_boot import _ntff_profile_via_ctypes
        hooks_mod.set_axon_ntff_profile_hook(
            _ntff_profile_via_ctypes("/opt/axon/libaxon_pjrt.so"))
    except Exception:
        pass


def _layernorm(nc, pool, pstat, pmm, small, ones128b, ones1, eps_t, x, w_pc, b_pc, out_bf, nm):
    """LN over d (partitions x chunks) of x [128, DC, 512] fp32 -> out_bf bf16."""
    xbf = pool.tile([P, DC, TPC], bf16, tag="xbf", name=f"xbf_{nm}")
    sqbf = pool.tile([P, DC, TPC], bf16, tag="sqbf", name=f"sqbf_{nm}")
    nc.vector.tensor_copy(xbf[:], x[:])
    nc.vector.tensor_mul(sqbf[:], xbf[:], xbf[:])
    sx = pstat.tile([1, TPC], f32, tag="stat", name=f"sx_{nm}")
    sq = pstat.tile([1, TPC], f32, tag="stat", name=f"sq_{nm}")
    for c in range(DC):
        nc.tensor.matmul(sx[:], ones128b[:], xbf[:, c, :], start=(c == 0), stop=(c == DC - 1))
    for c in range(DC):
        nc.tensor.matmul(sq[:], ones128b[:], sqbf[:, c, :], start=(c == 0), stop=(c == DC - 1))
    mu = small.tile([1, TPC], f32, tag="sm", name=f"mu_{nm}")
    ex2 = small.tile([1, TPC], f32, tag="sm", name=f"ex2_{nm}")
    nc.vector.tensor_scalar_mul(mu[:], sx[:], 1.0 / D)
    nc.vector.tensor_scalar_mul(ex2[:], sq[:], 1.0 / D)
    var = small.tile([1, TPC], f32, tag="sm", name=f"var_{nm}")
    nc.vector.tensor_mul(var[:], mu[:], mu[:])
    nc.vector.tensor_sub(var[:], ex2[:], var[:])
    nc.scalar.activation(var[:], var[:], AF.Sqrt, bias=eps_t[:], scale=1.0)
    rstd = small.tile([1, TPC], f32, tag="sm", name=f"rstd_{nm}")
    nc.vector.reciprocal(rstd[:], var[:])
    murstd = small.tile([1, TPC], f32, tag="sm", name=f"murstd_{nm}")
    nc.vector.tensor_mul(murstd[:], mu[:], rstd[:])
    rsb = pmm.tile([P, TPC], f32, tag="mm", name=f"rsb_{nm}")
    msb = pmm.tile([P, TPC], f32, tag="mm", name=f"msb_{nm}")
    nc.tensor.matmul(rsb[:], ones1[:], rstd[:], start=True, stop=True)
    nc.tensor.matmul(msb[:], ones1[:], murstd[:], start=True, stop=True)
    nc.vector.tensor_mul(out_bf[:], x[:], rsb[:, None, :].to_broadcast([P, DC, TPC]))
    nc.vector.tensor_sub(out_bf[:], out_bf[:], msb[:, None, :].to_broadcast([P, DC, TPC]))
    for c in range(DC):
        nc.vector.scalar_tensor_tensor(
            out_bf[:, c, :], out_bf[:, c, :], w_pc[:, c], b_pc[:, c].to_broadcast([P, TPC]),
            op0=OP.mult, op1=OP.add)


def _build():
    nc = bacc.Bacc(None, target_bir_lowering=False, debug=False)

    xembT = nc.dram_tensor("xembT", [D, TPC], f32, kind="ExternalInput")
    wq = nc.dram_tensor("wq", [L, P, DC, D], bf16, kind="ExternalInput")
    wk = nc.dram_tensor("wk", [L, P, DC, D], bf16, kind="ExternalInput")
    wv = nc.dram_tensor("wv", [L, P, DC, D], bf16, kind="ExternalInput")
    wo = nc.dram_tensor("wo", [L, P, DC, D], bf16, kind="ExternalInput")
    w1 = nc.dram_tensor("w1", [L, FC, P, DC, P], bf16, kind="ExternalInput")
    w2 = nc.dram_tensor("w2", [L, 4, DC, P, 8, P], bf16, kind="ExternalInput")
    wlm = nc.dram_tensor("wlm", [NVC, P, DC, 512], bf16, kind="ExternalInput")
    ln1w = nc.dram_tensor("ln1w", [L, P, DC], f32, kind="ExternalInput")
    ln1b = nc.dram_tensor("ln1b", [L, P, DC], f32, kind="ExternalInput")
    ln2w = nc.dram_tensor("ln2w", [L, P, DC], f32, kind="ExternalInput")
    ln2b = nc.dram_tensor("ln2b", [L, P, DC], f32, kind="ExternalInput")
    lnfw = nc.dram_tensor("lnfw", [P, DC], f32, kind="ExternalInput")
    lnfb = nc.dram_tensor("lnfb", [P, DC], f32, kind="ExternalInput")
    bo_d = nc.dram_tensor("bo", [L, P, DC], f32, kind="ExternalInput")
    b1_d = nc.dram_tensor("b1", [L, P, FC], f32, kind="ExternalInput")
    b2_d = nc.dram_tensor("b2", [L, P, DC], f32, kind="ExternalInput")
    blm_d = nc.dram_tensor("blm", [V], f32, kind="ExternalInput")
    mask_d = nc.dram_tensor("mask", [P, 2 * DC // 2, TPC], bf16, kind="ExternalInput")
    out_d = nc.dram_tensor("out", [TPC, V], f32, kind="ExternalOutput")

    kv_loc = nc.dram_tensor("kv_loc", [KV_SZ], bf16)
    kv_gat = nc.dram_tensor("kv_gat", [2, KV_SZ], bf16)
    groups = [[0, 1], [2, 3], [4, 5], [6, 7]]

    with tile.TileContext(nc) as tc:
        with (
            tc.tile_pool(name="pool", bufs=1) as pool,
            tc.tile_pool(name="wpool", bufs=2) as wpool,
            tc.tile_pool(name="abf", bufs=4) as abf,
            tc.tile_pool(name="sexp_p", bufs=2) as sexp_p,
            tc.tile_pool(name="small", bufs=5) as small,
            tc.tile_pool(name="lnp", bufs=4) as lnp,
            tc.tile_pool(name="outp", bufs=3) as outp,
            tc.tile_pool(name="pmm", bufs=6, space="PSUM") as pmm,
            tc.tile_pool(name="pstat", bufs=2, space="PSUM") as pstat,
        ):
            # ---- persistent tiles
            x = pool.tile([P, DC, TPC], f32, name="x")
            kfull = pool.tile([P, 2, DC, TPC], bf16, name="kfull")
            vfull = pool.tile([P, 2, 4, VW], bf16, name="vfull")
            mask = pool.tile([P, DC, TPC], bf16, name="mask")
            ones128b = pool.tile([P, 1], bf16, name="ones128b")
            ones1 = pool.tile([1, P], f32, name="ones1")
            nc.vector.memset(ones128b[:], 1.0)
            nc.vector.memset(ones1[:], 1.0)
            eps_t = pool.tile([1, 1], f32, name="eps_t")
            nc.vector.memset(eps_t[:], EPS)
            nc.sync.dma_start(mask[:], mask_d[:])
            nc.sync.dma_start(x[:], xembT.rearrange("(c p) t -> p c t", p=P))
            r = pool.tile([P, 8, TPC], bf16, name="r")

            def psum_mm(name):
                return pmm.tile([P, TPC], f32, tag="mm", name=name)

            def ln(xin, w_pc, b_pc, out_bf, nm):
                _layernorm(nc, pool, pstat, pmm, small, ones128b, ones1, eps_t,
                           xin, w_pc, b_pc, out_bf, nm)

            def ln_params(wd, bd, li, nm):
                wt = lnp.tile([P, DC, 1], f32, tag="lnw", name=f"lnw_{nm}")
                bt = lnp.tile([P, DC, 1], f32, tag="lnb", name=f"lnb_{nm}")
                src_w = wd[li] if li is not None else wd
                src_b = bd[li] if li is not None else bd
                nc.sync.dma_start(wt[:], src_w[:, :, None])
                nc.sync.dma_start(bt[:], src_b[:, :, None])
                return wt, bt

            for li in range(L):
                # ---------- LN1 ----------
                w_pc, b_pc = ln_params(ln1w, ln1b, li, f"1_{li}")
                hbf = abf.tile([P, DC, TPC], bf16, tag="a", name=f"hbf_{li}")
                ln(x, w_pc, b_pc, hbf, f"l1_{li}")

                # ---------- K, V projections first (feed the collective) ----
                wk_t = wpool.tile([P, DC, D], bf16, tag="w", name=f"wk_{li}")
                nc.sync.dma_start(wk_t[:], wk[li])
                kst = abf.tile([P, DC, TPC], bf16, tag="a", name=f"kst_{li}")
                for m in range(DC):
                    ps = psum_mm(f"kps_{li}_{m}")
                    for c in range(DC):
                        nc.tensor.matmul(ps[:], wk_t[:, c, m * P:(m + 1) * P],
                                         hbf[:, c, :], start=(c == 0), stop=(c == DC - 1))
                    nc.scalar.activation(kst[:, m, :], ps[:], AF.Copy)

                wv_t = wpool.tile([P, DC, D], bf16, tag="w", name=f"wv_{li}")
                nc.sync.dma_start(wv_t[:], wv[li])
                vst = abf.tile([P, 4, VW], bf16, tag="a", name=f"vst_{li}")
                nc.vector.memset(vst[:], 1.0)
                for tc4 in range(4):
                    for mh in range(2):
                        ps = psum_mm(f"vps_{li}_{tc4}_{mh}")
                        for c in range(DC):
                            nc.tensor.matmul(
                                ps[:], hbf[:, c, tc4 * P:(tc4 + 1) * P],
                                wv_t[:, c, mh * 512:(mh + 1) * 512],
                                start=(c == 0), stop=(c == DC - 1))
                        dst = vst[:, tc4, :].rearrange("p (h e) -> p h e", e=HS + 1)
                        nc.vector.tensor_copy(
                            dst[:, mh * 8:(mh + 1) * 8, 0:HS],
                            ps[:].rearrange("p (h e) -> p h e", e=HS))
                # stage K/V to DRAM and gather
                nc.sync.dma_start(
                    kv_loc[0:K_SZ].rearrange("(p c t) -> p c t", c=DC, t=TPC), kst[:])
                nc.sync.dma_start(
                    kv_loc[K_SZ:KV_SZ].rearrange("(p c t) -> p c t", c=4, t=VW), vst[:])
                nc.gpsimd.collective_compute(
                    "AllGather", OP.bypass, replica_groups=groups,
                    ins=[kv_loc[:]], outs=[kv_gat[:]])

                # ---------- Q projection (overlaps the collective) --------
                wq_t = wpool.tile([P, DC, D], bf16, tag="w", name=f"wq_{li}")
                nc.sync.dma_start(wq_t[:], wq[li])
                qbf = abf.tile([P, DC, TPC], bf16, tag="a", name=f"qbf_{li}")
                for m in range(DC):
                    ps = psum_mm(f"qps_{li}_{m}")
                    for c in range(DC):
                        nc.tensor.matmul(ps[:], wq_t[:, c, m * P:(m + 1) * P],
                                         hbf[:, c, :], start=(c == 0), stop=(c == DC - 1))
                    nc.scalar.activation(qbf[:, m, :], ps[:], AF.Copy)

                # ---------- gathered KV back to SBUF ----------------------
                for sg in range(2):
                    nc.sync.dma_start(
                        kfull[:, sg], kv_gat[sg, 0:K_SZ].rearrange("(p c t) -> p c t", c=DC, t=TPC))
                    nc.sync.dma_start(
                        vfull[:, sg], kv_gat[sg, K_SZ:KV_SZ].rearrange("(p c t) -> p c t", c=4, t=VW))

                # ---------- attention ---------------------------------------
                obf = abf.tile([P, DC, TPC], bf16, tag="a", name=f"obf_{li}")
                for h in range(H):
                    hp = (h % 2) * HS
                    hc = h // 2
                    sexp = sexp_p.tile([P, DC, TPC], bf16, tag="sexp", name=f"sexp_{li}_{h}")
                    for kt in range(DC):
                        sl, tl = kt // 4, (kt % 4) * P
                        ps = psum_mm(f"sps_{li}_{h}_{kt}")
                        nc.tensor.matmul(
                            ps[:], kfull[hp:hp + HS, sl, hc, tl:tl + P],
                            qbf[hp:hp + HS, hc, :], start=True, stop=True)
                        nc.scalar.activation(sexp[:, kt, :], ps[:], AF.Exp, scale=HS ** -0.5)
                    nc.vector.tensor_mul(sexp[:], sexp[:], mask[:])
                    ops = psum_mm(f"ops_{li}_{h}")
                    for kt in range(DC):
                        nc.tensor.matmul(
                            ops[0:HS + 1, :], vfull[:, kt // 4, kt % 4, h * 65:h * 65 + 65],
                            sexp[:, kt, :], start=(kt == 0), stop=(kt == DC - 1))
                    rc = small.tile([1, TPC], f32, tag="rcb", name=f"rc_{li}_{h}")
                    nc.vector.reciprocal(rc[:], ops[HS:HS + 1, :])
                    bc = psum_mm(f"bcp_{li}_{h}")
                    nc.tensor.matmul(bc[0:HS, :], ones1[:, 0:HS], rc[:], start=True, stop=True)
                    bcs = small.tile([HS, TPC], f32, tag="rcb", name=f"bcs_{li}_{h}")
                    nc.vector.tensor_copy(bcs[:], bc[0:HS, :])
                    nc.vector.tensor_mul(obf[hp:hp + HS, hc, :], ops[0:HS, :], bcs[:])

                # ---------- output projection + residual --------------------
                wo_t = wpool.tile([P, DC, D], bf16, tag="w", name=f"wo_{li}")
                nc.sync.dma_start(wo_t[:], wo[li])
                bo_t = lnp.tile([P, DC, 1], f32, tag="bias", name=f"bo_{li}")
                nc.sync.dma_start(bo_t[:], bo_d[li][:, :, None])
                for m in range(DC):
                    ps = psum_mm(f"ops2_{li}_{m}")
                    for c in range(DC):
                        nc.tensor.matmul(ps[:], wo_t[:, c, m * P:(m + 1) * P],
                                         obf[:, c, :], start=(c == 0), stop=(c == DC - 1))
                    nc.vector.scalar_tensor_tensor(
                        x[:, m, :], ps[:], bo_t[:, m], x[:, m, :], op0=OP.add, op1=OP.add)

                # ---------- LN2 + MLP ----------------------------------------
                w_pc2, b_pc2 = ln_params(ln2w, ln2b, li, f"2_{li}")
                h2 = abf.tile([P, DC, TPC], bf16, tag="a", name=f"h2_{li}")
                ln(x, w_pc2, b_pc2, h2, f"l2_{li}")

                b1_t = lnp.tile([P, FC, 1], f32, tag="b1", name=f"b1_{li}")
                nc.sync.dma_start(b1_t[:], b1_d[li][:, :, None])
                b2_t = lnp.tile([P, DC, 1], f32, tag="bias", name=f"b2_{li}")
                nc.sync.dma_start(b2_t[:], b2_d[li][:, :, None])
                for qr in range(4):
                    for mfl in range(8):
                        mf = qr * 8 + mfl
                        w1_t = wpool.tile([P, DC, P], bf16, tag="w1", name=f"w1_{li}_{mf}")
                        nc.sync.dma_start(w1_t[:], w1[li, mf])
                        ps = psum_mm(f"mps_{li}_{mf}")
                        for c in range(DC):
                            nc.tensor.matmul(ps[:], w1_t[:, c, :], h2[:, c, :],
                                             start=(c == 0), stop=(c == DC - 1))
                        nc.scalar.activation(r[:, mfl, :], ps[:], AF.Relu, bias=b1_t[:, mf], scale=1.0)
                    for m in range(DC):
                        w2_t = wpool.tile([P, 8, P], bf16, tag="w2", name=f"w2_{li}_{qr}_{m}")
                        nc.sync.dma_start(w2_t[:], w2[li, qr, m])
                        ps = psum_mm(f"m2ps_{li}_{qr}_{m}")
                        for c in range(8):
                            nc.tensor.matmul(ps[:], w2_t[:, c, :], r[:, c, :],
                                             start=(c == 0), stop=(c == 7))
                        if qr == 0:
                            nc.vector.scalar_tensor_tensor(
                                x[:, m, :], ps[:], b2_t[:, m], x[:, m, :], op0=OP.add, op1=OP.add)
                        else:
                            nc.vector.tensor_add(x[:, m, :], x[:, m, :], ps[:])

            # ---------- final LN + LM head ----------------------------------
            w_pcf, b_pcf = ln_params(lnfw, lnfb, None, "f")
            xf = abf.tile([P, DC, TPC], bf16, tag="a", name="xf")
            ln(x, w_pcf, b_pcf, xf, "lf")

            for vc in range(NVC):
                nv = min(512, V - vc * 512)
                wl_t = wpool.tile([P, DC, 512], bf16, tag="w", name=f"wlm_{vc}")
                nc.sync.dma_start(wl_t[:], wlm[vc])
                bl = small.tile([1, 512], f32, tag="rcb", name=f"bl_{vc}")
                nc.sync.dma_start(bl[:, 0:nv], blm_d[None, vc * 512:vc * 512 + nv])
                bcp = psum_mm(f"blmp_{vc}")
                nc.tensor.matmul(bcp[:, 0:nv], ones1[:], bl[:, 0:nv], start=True, stop=True)
                bls = outp.tile([P, 512], f32, tag="o", name=f"bls_{vc}")
                nc.vector.tensor_copy(bls[:, 0:nv], bcp[:, 0:nv])
                for tc4 in range(4):
                    ps = psum_mm(f"lmps_{vc}_{tc4}")
                    for c in range(DC):
                        nc.tensor.matmul(ps[:, 0:nv], xf[:, c, tc4 * P:(tc4 + 1) * P],
                                         wl_t[:, c, 0:nv], start=(c == 0), stop=(c == DC - 1))
                    ot = outp.tile([P, 512], f32, tag="o", name=f"ot_{vc}_{tc4}")
                    nc.vector.tensor_add(ot[:, 0:nv], ps[:, 0:nv], bls[:, 0:nv])
                    nc.sync.dma_start(
                        out_d[tc4 * P:(tc4 + 1) * P, vc * 512:vc * 512 + nv], ot[:, 0:nv])

    nc.compile()
    return nc


def kernel(**inputs):
    global LAST_EXEC_NS
    _install_ntff_hook()
    if "nc" not in _CACHE:
        _CACHE["nc"] = _build()
    nc = _CACHE["nc"]

    gi = {k: np.asarray(v) for k, v in inputs.items()}
    idx = gi["idx"].astype(np.int64)
    xemb = gi["wte"][idx] + gi["wpe"][:T][None, :, :]      # [B, T, D] fp32

    def cast(a):
        return np.ascontiguousarray(a.astype(ml_dtypes.bfloat16))

    def pack_sq(w):   # [L, 1024, N] -> [L, 128, 8, N]
        Lw, Kw, Nw = w.shape
        return np.ascontiguousarray(
            w.reshape(Lw, DC, P, Nw).transpose(0, 2, 1, 3).astype(ml_dtypes.bfloat16))

    w1p = gi["w1"].reshape(L, DC, P, FC, P).transpose(0, 3, 2, 1, 4)   # [L,FC,P,DC,P]
    w1p = np.ascontiguousarray(w1p.astype(ml_dtypes.bfloat16))
    w2p = gi["w2"].reshape(L, 4, 8, P, DC, P).transpose(0, 1, 4, 3, 2, 5)  # [L,4,DC,P,8,P]
    w2p = np.ascontiguousarray(w2p.astype(ml_dtypes.bfloat16))
    wlmp = np.zeros((D, NVC * 512), np.float32)
    wlmp[:, :V] = gi["wlm"]
    wlmp = wlmp.reshape(DC, P, NVC, 512).transpose(2, 1, 0, 3)         # [NVC,P,DC,512]
    wlmp = np.ascontiguousarray(wlmp.astype(ml_dtypes.bfloat16))

    def packv(v):  # [.., N] -> [.., P, N//P] (chunk-major per partition)
        v = np.asarray(v, np.float32)
        nch = v.shape[-1] // P
        return np.ascontiguousarray(
            v.reshape(v.shape[:-1] + (nch, P)).swapaxes(-1, -2))

    shared = dict(
        wq=pack_sq(gi["wq"]), wk=pack_sq(gi["wk"]), wv=pack_sq(gi["wv"]), wo=pack_sq(gi["wo"]),
        w1=w1p, w2=w2p, wlm=wlmp,
        ln1w=packv(gi["ln1_w"]), ln1b=packv(gi["ln1_b"]),
        ln2w=packv(gi["ln2_w"]), ln2b=packv(gi["ln2_b"]),
        lnfw=packv(gi["lnf_w"]), lnfb=packv(gi["lnf_b"]),
        bo=packv(gi["bo"]), b1=packv(gi["b1"]), b2=packv(gi["b2"]),
        blm=np.ascontiguousarray(gi["blm"], np.float32),
    )

    in_maps = []
    for c in range(8):
        b, half = c // 2, c % 2
        q0 = half * TPC
        sl = slice(q0, q0 + TPC)
        m = np.zeros((P, DC, TPC), np.float32)
        k_abs = np.arange(P)[:, None] + (np.arange(DC) * P)[None, :]   # [P, DC]
        q_abs = q0 + np.arange(TPC)
        m[:] = (k_abs[:, :, None] <= q_abs[None, None, :]).astype(np.float32)
        im = dict(shared)
        im["xembT"] = np.ascontiguousarray(xemb[b, sl].T, dtype=np.float32)
        im["mask"] = m.astype(ml_dtypes.bfloat16)
        in_maps.append(im)

    res = run_bass_kernel_spmd(nc, in_maps, list(range(8)),
                               trace=bool(os.environ.get("BASS_TRACE")))
    LAST_EXEC_NS = res.exec_time_ns

    out = np.empty((B, T, V), np.float32)
    for c in range(8):
        b, half = c // 2, c % 2
        out[b, half * TPC:(half + 1) * TPC] = res.results[c]["out"]
    return out



# revision 15
# speedup vs baseline: 1.4375x; 1.4375x over previous
"""GPT-2 (L=8, D=1024, H=16, V=50257, B=4, T=1024) forward on 8 TRN2 NeuronCores.

v2 design:
- Core c handles batch b=c//2; EVEN cores take sequence half 1 (tokens 512-1023),
  ODD cores half 0.  With AllGather slot order = rank order, slot 1 of the
  gathered buffer is always the half-0 K/V, so the "remote" attention reads a
  fixed slot on every core (SPMD-uniform): real work on even cores, nulled on
  odd cores via an exp bias of -30000 (exp -> 0).
- Attention split local/remote: local scores/AV use the freshly projected K/V
  straight from SBUF (causal lower-tri only: shrinking-N matmuls + one 128x128
  tri mask per diagonal block).  Remote K/V travel through two fp8(e3m4)
  AllGathers (K first, then V) that overlap Q projection + local attention.
- LN2/LNf affine folded into w1/b1 and wlm/blm on the host (exact).
- LM head vocab-major: out[vocab,tok] so blm rides the PSUM->SBUF evacuation
  as a per-partition bias; bf16 output, host transposes/upcasts.
- reciprocal_approx_fast for softmax denominators and LN rstd.
"""

import os
import sys
import types

import numpy as np
import ml_dtypes

import concourse.bass as bass
import concourse.mybir as mybir
import concourse.tile as tile
from concourse import bacc
from concourse.bass_utils import run_bass_kernel_spmd

f32 = mybir.dt.float32
bf16 = mybir.dt.bfloat16
fp8 = mybir.dt.float8e3
AF = mybir.ActivationFunctionType
OP = mybir.AluOpType

L, D, H, V, DFF = 8, 1024, 16, 50257, 4096
HS = D // H          # 64
B, T = 4, 1024
TPC = 512            # tokens per core
P = 128
DC = D // P          # 8 d-chunks
FC = DFF // P        # 32 dff-chunks
NV = 50688           # padded vocab (396 * 128)
NVC = NV // P        # 396 vocab chunks
NVG = NVC // 4       # 99 vocab groups of 4 chunks
EPS = 1e-5

KSZ = D * TPC                  # K staging elems per core
VW = H * (HS + 1)              # 1040
VSZ = 4 * P * VW               # V_aug staging elems per core

LAST_EXEC_NS = None
_CACHE = {}


def _install_ntff_hook():
    """Provide antenv.axon_hooks if the image lacks it, so trace=True works."""
    try:
        import antenv
        try:
            from antenv import axon_hooks  # noqa: F401
            return
        except ImportError:
            pass
        hooks_mod = types.ModuleType("antenv.axon_hooks")
        _hook = [None]
        hooks_mod.set_axon_ntff_profile_hook = lambda h: _hook.__setitem__(0, h)
        hooks_mod.get_axon_ntff_profile_hook = lambda: _hook[0]
        sys.modules["antenv.axon_hooks"] = hooks_mod
        antenv.axon_hooks = hooks_mod
        from trn_agent_boot.trn_boot import _ntff_profile_via_ctypes
        hooks_mod.set_axon_ntff_profile_hook(
            _ntff_profile_via_ctypes("/opt/axon/libaxon_pjrt.so"))
    except Exception:
        pass


def _build():
    nc = bacc.Bacc(None, target_bir_lowering=False, debug=False)

    xembT = nc.dram_tensor("xembT", [D, TPC], f32, kind="ExternalInput")
    wq = nc.dram_tensor("wq", [L, P, DC, D], bf16, kind="ExternalInput")
    wk = nc.dram_tensor("wk", [L, P, DC, D], bf16, kind="ExternalInput")
    wv = nc.dram_tensor("wv", [L, P, DC, D], bf16, kind="ExternalInput")
    wo = nc.dram_tensor("wo", [L, P, DC, D], bf16, kind="ExternalInput")
    w1 = nc.dram_tensor("w1", [L, 8, P, 4, DC, P], bf16, kind="ExternalInput")
    w2 = nc.dram_tensor("w2", [L, 4, P, DC, 8, P], bf16, kind="ExternalInput")
    wlm = nc.dram_tensor("wlm", [NVG, P, 4, DC, P], bf16, kind="ExternalInput")
    blm_d = nc.dram_tensor("blm", [P, NVC], f32, kind="ExternalInput")
    pp_d = nc.dram_tensor("pp", [L, P, 64], f32, kind="ExternalInput")
    tri_d = nc.dram_tensor("tri", [P, P], bf16, kind="ExternalInput")
    lmz_d = nc.dram_tensor("lmz", [P, 1], f32, kind="ExternalInput")
    out_d = nc.dram_tensor("out", [NV, TPC], bf16, kind="ExternalOutput")

    kK_loc = nc.dram_tensor("kK_loc", [KSZ], fp8)
    kK_gat = nc.dram_tensor("kK_gat", [2, KSZ], fp8)
    kV_loc = nc.dram_tensor("kV_loc", [VSZ], fp8)
    kV_gat = nc.dram_tensor("kV_gat", [2, VSZ], fp8)
    groups = [[0, 1], [2, 3], [4, 5], [6, 7]]

    with tile.TileContext(nc) as tc:
        with (
            tc.tile_pool(name="pers", bufs=1) as pers,
            tc.tile_pool(name="wpool", bufs=2) as wpool,
            tc.tile_pool(name="act", bufs=3) as act,
            tc.tile_pool(name="lnp", bufs=1) as lnp,
            tc.tile_pool(name="kvp", bufs=2) as kvp,
            tc.tile_pool(name="sexp_p", bufs=2) as sexp_p,
            tc.tile_pool(name="small", bufs=6) as small,
            tc.tile_pool(name="outp", bufs=3) as outp,
            tc.tile_pool(name="ppool", bufs=2) as ppool,
            tc.tile_pool(name="mlpr", bufs=2) as mlpr,
            tc.tile_pool(name="psum", bufs=2, space="PSUM") as psum,
        ):
            # ---- persistent tiles
            x = pers.tile([P, DC, TPC], f32, name="x")
            ones128b = pers.tile([P, 1], bf16, name="ones128b")
            ones1 = pers.tile([1, P], f32, name="ones1")
            nc.vector.memset(ones128b[:], 1.0)
            nc.vector.memset(ones1[:], 1.0)
            eps_t = pers.tile([1, 1], f32, name="eps_t")
            nc.vector.memset(eps_t[:], EPS)
            tri_t = pers.tile([P, P], bf16, name="tri_t")
            nc.sync.dma_start(tri_t[:], tri_d[:])
            lmz_t = pers.tile([P, 1], f32, name="lmz_t")
            nc.sync.dma_start(lmz_t[:], lmz_d[:])
            bl_t = pers.tile([P, NVC], f32, name="bl_t")
            nc.sync.dma_start(bl_t[:], blm_d[:])
            nc.sync.dma_start(x[:], xembT.rearrange("(c p) t -> p c t", p=P))

            def ln(xin, w_ap, b_ap, out_bf, nm):
                """LN over d of xin [128, DC, 512] f32 -> out_bf bf16.
                w_ap/b_ap: [P, DC]-sliceable APs or None (affine folded)."""
                xbf = lnp.tile([P, DC, TPC], bf16, tag="lnx", name=f"xbf_{nm}")
                nc.vector.tensor_copy(xbf[:], xin[:])
                sx = psum.tile([1, TPC], f32, tag="stat", name=f"sx_{nm}")
                sq = psum.tile([1, TPC], f32, tag="stat", name=f"sq_{nm}")
                for c in range(DC):
                    nc.tensor.matmul(sx[:], ones128b[:], xbf[:, c, :],
                                     start=(c == 0), stop=(c == DC - 1))
                for c in range(DC):
                    sqb = lnp.tile([P, TPC], bf16, tag="lnq", bufs=2,
                                   name=f"sqb_{nm}_{c}")
                    nc.vector.tensor_mul(sqb[:], xbf[:, c, :], xbf[:, c, :])
                    nc.tensor.matmul(sq[:], ones128b[:], sqb[:],
                                     start=(c == 0), stop=(c == DC - 1))
                mu = small.tile([1, TPC], f32, tag="sm", name=f"mu_{nm}")
                ex2 = small.tile([1, TPC], f32, tag="sm", name=f"ex2_{nm}")
                nc.vector.tensor_scalar_mul(mu[:], sx[:], 1.0 / D)
                nc.vector.tensor_scalar_mul(ex2[:], sq[:], 1.0 / D)
                var = small.tile([1, TPC], f32, tag="sm", name=f"var_{nm}")
                nc.vector.tensor_mul(var[:], mu[:], mu[:])
                nc.vector.tensor_sub(var[:], ex2[:], var[:])
                nc.scalar.activation(var[:], var[:], AF.Sqrt, bias=eps_t[:], scale=1.0)
                rstd = small.tile([1, TPC], f32, tag="sm", name=f"rstd_{nm}")
                nc.vector.reciprocal_approx_fast(rstd[:], var[:])
                murstd = small.tile([1, TPC], f32, tag="sm", name=f"murstd_{nm}")
                nc.vector.tensor_mul(murstd[:], mu[:], rstd[:])
                rsb = psum.tile([P, TPC], f32, tag="mm", name=f"rsb_{nm}")
                msb = psum.tile([P, TPC], f32, tag="mm", name=f"msb_{nm}")
                nc.tensor.matmul(rsb[:], ones1[:], rstd[:], start=True, stop=True)
                nc.tensor.matmul(msb[:], ones1[:], murstd[:], start=True, stop=True)
                nc.vector.tensor_mul(out_bf[:], xin[:],
                                     rsb[:, None, :].to_broadcast([P, DC, TPC]))
                nc.vector.tensor_sub(out_bf[:], out_bf[:],
                                     msb[:, None, :].to_broadcast([P, DC, TPC]))
                if w_ap is not None:
                    for c in range(DC):
                        nc.vector.scalar_tensor_tensor(
                            out_bf[:, c, :], out_bf[:, c, :], w_ap[:, c:c + 1],
                            b_ap[:, c:c + 1].to_broadcast([P, TPC]),
                            op0=OP.mult, op1=OP.add)

            for li in range(L):
                pp_t = ppool.tile([P, 64], f32, tag="pp", name=f"pp_{li}")
                nc.sync.dma_start(pp_t[:], pp_d[li])

                # ---------- LN1 (affine kept on device) ----------
                hbf = act.tile([P, DC, TPC], bf16, tag="a", name=f"hbf_{li}")
                ln(x, pp_t[:, 0:DC], pp_t[:, DC:2 * DC], hbf, f"l1_{li}")

                # ---------- K projection -> fp8, stage, AllGather ----------
                kst = kvp.tile([P, DC, TPC], fp8, tag="k", bufs=1, name=f"kst_{li}")
                for hf in range(2):
                    wk_t = wpool.tile([P, DC, 512], bf16, tag="w", name=f"wk_{li}_{hf}")
                    nc.sync.dma_start(wk_t[:], wk[li, :, :, hf * 512:(hf + 1) * 512])
                    for m4 in range(4):
                        m = hf * 4 + m4
                        ps = psum.tile([P, TPC], f32, tag="mm", name=f"kps_{li}_{m}")
                        for c in range(DC):
                            nc.tensor.matmul(ps[:], wk_t[:, c, m4 * P:(m4 + 1) * P],
                                             hbf[:, c, :], start=(c == 0), stop=(c == DC - 1))
                        nc.scalar.activation(kst[:, m, :], ps[:], AF.Copy)
                nc.sync.dma_start(
                    kK_loc.rearrange("(p c t) -> p c t", c=DC, t=TPC), kst[:])
                nc.gpsimd.collective_compute(
                    "AllGather", OP.bypass, replica_groups=groups,
                    ins=[kK_loc[:]], outs=[kK_gat[:]])

                # ---------- V projection -> fp8, stage, AllGather ----------
                vst = kvp.tile([P, 4, VW], fp8, tag="v", bufs=1, name=f"vst_{li}")
                nc.vector.memset(vst[:], 1.0)
                for mh in range(2):
                    wv_t = wpool.tile([P, DC, 512], bf16, tag="w", name=f"wv_{li}_{mh}")
                    nc.sync.dma_start(wv_t[:], wv[li, :, :, mh * 512:(mh + 1) * 512])
                    for tc4 in range(4):
                        ps = psum.tile([P, TPC], f32, tag="mm", name=f"vps_{li}_{tc4}_{mh}")
                        for c in range(DC):
                            nc.tensor.matmul(
                                ps[:], hbf[:, c, tc4 * P:(tc4 + 1) * P],
                                wv_t[:, c, :],
                                start=(c == 0), stop=(c == DC - 1))
                        dst = vst[:, tc4, :].rearrange("p (h e) -> p h e", e=HS + 1)
                        nc.vector.tensor_copy(
                            dst[:, mh * 8:(mh + 1) * 8, 0:HS],
                            ps[:].rearrange("p (h e) -> p h e", e=HS))
                nc.sync.dma_start(
                    kV_loc.rearrange("(p c t) -> p c t", c=4, t=VW), vst[:])
                nc.gpsimd.collective_compute(
                    "AllGather", OP.bypass, replica_groups=groups,
                    ins=[kV_loc[:]], outs=[kV_gat[:]])

                # ---------- Q projection (overlaps the collectives) ----------
                qbf = act.tile([P, DC, TPC], bf16, tag="a", name=f"qbf_{li}")
                for hf in range(2):
                    wq_t = wpool.tile([P, DC, 512], bf16, tag="w", name=f"wq_{li}_{hf}")
                    nc.sync.dma_start(wq_t[:], wq[li, :, :, hf * 512:(hf + 1) * 512])
                    for m4 in range(4):
                        m = hf * 4 + m4
                        ps = psum.tile([P, TPC], f32, tag="mm", name=f"qps_{li}_{m}")
                        for c in range(DC):
                            nc.tensor.matmul(ps[:], wq_t[:, c, m4 * P:(m4 + 1) * P],
                                             hbf[:, c, :], start=(c == 0), stop=(c == DC - 1))
                        nc.scalar.activation(qbf[:, m, :], ps[:], AF.Copy)

                # ---------- remote K/V from gathered slot 1 (fixed) ----------
                krem = kvp.tile([P, DC, TPC], fp8, tag="kr", bufs=1, name=f"krem_{li}")
                nc.sync.dma_start(
                    krem[:], kK_gat[1].rearrange("(p c t) -> p c t", c=DC, t=TPC))
                vrem = kvp.tile([P, 4, VW], fp8, tag="vr", bufs=1, name=f"vrem_{li}")
                nc.sync.dma_start(
                    vrem[:], kV_gat[1].rearrange("(p c t) -> p c t", c=4, t=VW))

                # ---------- attention ----------
                obf = act.tile([P, DC, TPC], bf16, tag="a", name=f"obf_{li}")
                scale = HS ** -0.5
                for h in range(H):
                    hp = (h % 2) * HS
                    hc = h // 2
                    # local: causal lower-tri over own 512 keys
                    sl = sexp_p.tile([P, 4, TPC], bf16, tag="sl", name=f"sl_{li}_{h}")
                    for kt in range(4):
                        c0 = kt * P
                        N = TPC - c0
                        ss = psum.tile([P, TPC], f32, tag="sc", name=f"ssl_{li}_{h}_{kt}")
                        nc.tensor.matmul(
                            ss[:, 0:N], kst[hp:hp + HS, hc, kt * P:(kt + 1) * P],
                            qbf[hp:hp + HS, hc, c0:TPC], start=True, stop=True)
                        nc.scalar.activation(sl[:, kt, c0:TPC], ss[:, 0:N],
                                             AF.Exp, scale=scale)
                        nc.vector.tensor_mul(sl[:, kt, c0:c0 + P],
                                             sl[:, kt, c0:c0 + P], tri_t[:])
                    av = psum.tile([P, TPC], f32, tag="av", name=f"av_{li}_{h}")
                    for kt in range(4):
                        c0 = kt * P
                        nc.tensor.matmul(
                            av[0:HS + 1, c0:TPC], vst[:, kt, h * 65:h * 65 + 65],
                            sl[:, kt, c0:TPC], start=(kt == 0), stop=False)
                    # remote: full 512 keys (nulled on odd cores via exp bias)
                    sr = sexp_p.tile([P, 4, TPC], bf16, tag="sr", name=f"sr_{li}_{h}")
                    for kt in range(4):
                        ss = psum.tile([P, TPC], f32, tag="sc", name=f"ssr_{li}_{h}_{kt}")
                        nc.tensor.matmul(
                            ss[:], krem[hp:hp + HS, hc, kt * P:(kt + 1) * P],
                            qbf[hp:hp + HS, hc, :], start=True, stop=True)
                        nc.scalar.activation(sr[:, kt, :], ss[:], AF.Exp,
                                             bias=lmz_t[:], scale=scale)
                    for kt in range(4):
                        nc.tensor.matmul(
                            av[0:HS + 1, :], vrem[:, kt, h * 65:h * 65 + 65],
                            sr[:, kt, :], start=False, stop=(kt == 3))
                    # normalize by the ones-row denominator (bounce the
                    # partition-64 PSUM row through SBUF: the custom-DVE recip
                    # does not honor base_partition offsets)
                    den_s = small.tile([1, TPC], f32, tag="rc", name=f"den_{li}_{h}")
                    nc.scalar.activation(den_s[:], av[HS:HS + 1, :], AF.Copy)
                    rc = small.tile([1, TPC], f32, tag="rc", name=f"rc_{li}_{h}")
                    nc.vector.reciprocal_approx_fast(rc[:], den_s[:])
                    bc = psum.tile([P, TPC], f32, tag="sc", name=f"bc_{li}_{h}")
                    nc.tensor.matmul(bc[0:HS, :], ones1[:, 0:HS], rc[:],
                                     start=True, stop=True)
                    bcs = small.tile([HS, TPC], bf16, tag="bcs", bufs=2,
                                     name=f"bcs_{li}_{h}")
                    nc.scalar.activation(bcs[:], bc[0:HS, :], AF.Copy)
                    nc.vector.tensor_mul(obf[hp:hp + HS, hc, :], av[0:HS, :],
                                         bcs[:])

                # ---------- output projection + residual ----------
                for hf in range(2):
                    wo_t = wpool.tile([P, DC, 512], bf16, tag="w", name=f"wo_{li}_{hf}")
                    nc.sync.dma_start(wo_t[:], wo[li, :, :, hf * 512:(hf + 1) * 512])
                    for m4 in range(4):
                        m = hf * 4 + m4
                        ps = psum.tile([P, TPC], f32, tag="mm", name=f"ops2_{li}_{m}")
                        for c in range(DC):
                            nc.tensor.matmul(ps[:], wo_t[:, c, m4 * P:(m4 + 1) * P],
                                             obf[:, c, :], start=(c == 0), stop=(c == DC - 1))
                        nc.vector.scalar_tensor_tensor(
                            x[:, m, :], ps[:], pp_t[:, 16 + m:17 + m], x[:, m, :],
                            op0=OP.add, op1=OP.add)

                # ---------- LN2 (affine folded into w1/b1) + MLP ----------
                h2 = act.tile([P, DC, TPC], bf16, tag="a", name=f"h2_{li}")
                ln(x, None, None, h2, f"l2_{li}")

                for qr in range(4):
                    rq = mlpr.tile([P, 8, TPC], bf16, tag="r", name=f"rq_{li}_{qr}")
                    for g2 in range(2):
                        g = qr * 2 + g2
                        w1_t = wpool.tile([P, 4, DC, P], bf16, tag="w1",
                                          name=f"w1_{li}_{g}")
                        nc.sync.dma_start(w1_t[:], w1[li, g])
                        for jj in range(4):
                            mf = g * 4 + jj
                            ps = psum.tile([P, TPC], f32, tag="mm", name=f"mps_{li}_{mf}")
                            for c in range(DC):
                                nc.tensor.matmul(ps[:], w1_t[:, jj, c, :], h2[:, c, :],
                                                 start=(c == 0), stop=(c == DC - 1))
                            nc.scalar.activation(rq[:, g2 * 4 + jj, :], ps[:], AF.Relu,
                                                 bias=pp_t[:, 32 + mf:33 + mf], scale=1.0)
                    w2_t = wpool.tile([P, DC, 8, P], bf16, tag="w2", name=f"w2_{li}_{qr}")
                    nc.sync.dma_start(w2_t[:], w2[li, qr])
                    for m in range(DC):
                        ps = psum.tile([P, TPC], f32, tag="mm", name=f"m2ps_{li}_{qr}_{m}")
                        for c in range(8):
                            nc.tensor.matmul(ps[:], w2_t[:, c, m, :], rq[:, c, :],
                                             start=(c == 0), stop=(c == 7))
                        if qr == 0:
                            nc.vector.scalar_tensor_tensor(
                                x[:, m, :], ps[:], pp_t[:, 24 + m:25 + m], x[:, m, :],
                                op0=OP.add, op1=OP.add)
                        else:
                            nc.vector.tensor_add(x[:, m, :], x[:, m, :], ps[:])

            # ---------- final LN (affine folded) + LM head ----------
            xf = act.tile([P, DC, TPC], bf16, tag="a", name="xf")
            ln(x, None, None, xf, "lf")

            for vg in range(NVG):
                wl_t = wpool.tile([P, 4, DC, P], bf16, tag="w1", name=f"wlm_{vg}")
                nc.sync.dma_start(wl_t[:], wlm[vg])
                for jj in range(4):
                    vc = vg * 4 + jj
                    ps = psum.tile([P, TPC], f32, tag="mm", name=f"lmps_{vc}")
                    for c in range(DC):
                        nc.tensor.matmul(ps[:], wl_t[:, jj, c, :], xf[:, c, :],
                                         start=(c == 0), stop=(c == DC - 1))
                    ot = outp.tile([P, TPC], bf16, tag="o", name=f"ot_{vc}")
                    nc.scalar.activation(ot[:], ps[:], AF.Identity,
                                         bias=bl_t[:, vc:vc + 1], scale=1.0)
                    nc.sync.dma_start(out_d[vc * P:(vc + 1) * P, :], ot[:])

    nc.compile()
    return nc


def kernel(**inputs):
    global LAST_EXEC_NS
    _install_ntff_hook()
    if "nc" not in _CACHE:
        _CACHE["nc"] = _build()
    nc = _CACHE["nc"]

    gi = {k: np.asarray(v) for k, v in inputs.items()}
    idx = gi["idx"].astype(np.int64)
    xemb = gi["wte"][idx] + gi["wpe"][:T][None, :, :]      # [B, T, D] fp32

    def pack_sq(w):   # [L, 1024, N] -> [L, 128, 8, N] bf16
        Lw, Kw, Nw = w.shape
        return np.ascontiguousarray(
            w.reshape(Lw, DC, P, Nw).transpose(0, 2, 1, 3).astype(ml_dtypes.bfloat16))

    def packv(v):  # [.., N] -> [.., P, N//P] chunk-major per partition
        v = np.asarray(v, np.float32)
        nch = v.shape[-1] // P
        return np.ascontiguousarray(
            v.reshape(v.shape[:-1] + (nch, P)).swapaxes(-1, -2))

    # ---- fold LN2 affine into w1/b1, LNf affine into wlm/blm (exact) ----
    w1f = gi["w1"].astype(np.float64) * gi["ln2_w"].astype(np.float64)[:, :, None]
    b1f = gi["b1"].astype(np.float64) + np.einsum(
        "ld,ldf->lf", gi["ln2_b"].astype(np.float64), gi["w1"].astype(np.float64))
    wlmf = gi["wlm"].astype(np.float64) * gi["lnf_w"].astype(np.float64)[:, None]
    blmf = gi["blm"].astype(np.float64) + gi["lnf_b"].astype(np.float64) @ \
        gi["wlm"].astype(np.float64)
    w1f = w1f.astype(np.float32)
    b1f = b1f.astype(np.float32)
    wlmf = wlmf.astype(np.float32)
    blmf = blmf.astype(np.float32)

    # w1: [L, D, DFF] -> [L, 8, P, 4, DC, P]
    a = w1f.reshape(L, DC, P, FC, P).transpose(0, 3, 2, 1, 4)   # [l, mf, p, c, f]
    a = a.reshape(L, 8, 4, P, DC, P).transpose(0, 1, 3, 2, 4, 5)
    w1p = np.ascontiguousarray(a.astype(ml_dtypes.bfloat16))
    # w2: [L, DFF, D] -> [L, 4, P, DC, 8, P]
    b = gi["w2"].reshape(L, 4, DC, P, DC, P).transpose(0, 1, 3, 2, 4, 5)
    w2p = np.ascontiguousarray(b.astype(ml_dtypes.bfloat16))
    # wlm: [D, V] -> padded vocab-major [NVG, P, 4, DC, P]
    wpad = np.zeros((D, NV), np.float32)
    wpad[:, :V] = wlmf
    cw = wpad.reshape(DC, P, NVC, P).transpose(2, 1, 0, 3)      # [vc, p, c, vf]
    cw = cw.reshape(NVG, 4, P, DC, P).transpose(0, 2, 1, 3, 4)  # [vg, p, j, c, vf]
    wlmp = np.ascontiguousarray(cw.astype(ml_dtypes.bfloat16))
    bpad = np.zeros((NV,), np.float32)
    bpad[:V] = blmf
    blp = np.ascontiguousarray(bpad.reshape(NVC, P).T)          # [P, NVC]

    # per-layer small params: [L, P, 64] = [ln1w | ln1b | bo | b2 | b1']
    pp = np.zeros((L, P, 64), np.float32)
    pp[:, :, 0:8] = packv(gi["ln1_w"])
    pp[:, :, 8:16] = packv(gi["ln1_b"])
    pp[:, :, 16:24] = packv(gi["bo"])
    pp[:, :, 24:32] = packv(gi["b2"])
    pp[:, :, 32:64] = packv(b1f)

    tri = np.ascontiguousarray(
        (np.arange(P)[:, None] <= np.arange(P)[None, :]).astype(ml_dtypes.bfloat16))

    shared = dict(
        wq=pack_sq(gi["wq"]), wk=pack_sq(gi["wk"]), wv=pack_sq(gi["wv"]),
        wo=pack_sq(gi["wo"]), w1=w1p, w2=w2p, wlm=wlmp, blm=blp, pp=pp, tri=tri,
    )

    in_maps = []
    for c in range(8):
        bb = c // 2
        half = 1 if c % 2 == 0 else 0          # even cores take tokens 512-1023
        q0 = half * TPC
        im = dict(shared)
        im["xembT"] = np.ascontiguousarray(xemb[bb, q0:q0 + TPC].T, dtype=np.float32)
        im["lmz"] = np.full((P, 1), 0.0 if half == 1 else -30000.0, np.float32)
        in_maps.append(im)

    res = run_bass_kernel_spmd(nc, in_maps, list(range(8)),
                               trace=bool(os.environ.get("BASS_TRACE")))
    LAST_EXEC_NS = res.exec_time_ns

    out = np.empty((B, T, V), np.float32)
    for c in range(8):
        bb = c // 2
        half = 1 if c % 2 == 0 else 0
        q0 = half * TPC
        out[bb, q0:q0 + TPC] = res.results[c]["out"][:V, :].T.astype(np.float32)
    return out
